# revision 19
# baseline (speedup 1.0000x reference)
"""Trainium2 Bass kernel for nn_InputRotationWrapper: y = WHT(x) @ W^T + b.

Algebraic fold: WHT (normalized Walsh-Hadamard along feature dim, H symmetric)
commutes into the weight: y = (x H) W^T = x (W H)^T.  The device runs a pure
GEMM  y = x @ Wr^T + b  with Wr = WHT(W) computed once on the host.

On top of the fold, one level of STRASSEN over 2x2x2 blocking of
(o, k, t) cuts the PE matmul count by 1/8 — the kernel is PE-streaming-bound
at fp16 (1 moving column/cycle), so this is a direct 12.5% win that neither
fp8 (accuracy: e4m3 x,W measures 3.8e-2 rel err vs the 2e-2 gate) nor uint8
(TRN2 silicon zeroes integer matmul products; probed via NEFF dtype patch)
can reach.

  C = Wr @ x^T = [[C11 C12],[C21 C22]],  A = Wr halves, B = x^T halves
  M1=(A11+A22)(B11+B22) M2=(A21+A22)B11 M3=A11(B12-B22) M4=A22(B21-B11)
  M5=(A11+A12)B22 M6=(A21-A11)(B11+B12) M7=(A12-A22)(B21+B22)
  C11=M1+M4-M5+M7  C12=M3+M5  C21=M2+M4  C22=M1-M2+M3+M6

All 7 A-combos (W-side) and 7 B-combos (x-side) are precomputed on the host
in f64/f32 and shipped as fp16: the device only runs products and cheap
recombines.  Per core (1024 tokens, data-parallel over 8 cores):

  - 7 x-combos resident in SBUF: [128p, 16c, 512t] fp16 each (14.7 MB)
  - W-combos streamed per (product j, o-block obp): [128p, 16c, 128o] fp16
  - 16 obp iterations x 7 products x 16-chunk PSUM accumulation
    = 1792 matmuls of 512 cols (vs 2048 classical) ~ 387 us PE wall
  - ScalarE evicts each product PSUM->SBUF fp16; VectorE recombines with
    scalar_tensor_tensor (bias fused via the per-partition scalar operand);
    outputs DMA per [128, 512] slice.  All hidden under PE time.

Startup mirrors the fp16 baseline: PE-clock warmup dummies, then a j-major
group over the first G o-blocks processed c-outer so every arriving x-combo
chunk immediately unlocks G matmuls while the DMA subsystem ramps.
"""
import sys

for _p in ("/opt/trn_rl_repo", "/root/.axon_site/_ro/trn_rl_repo"):
    if _p not in sys.path:
        sys.path.insert(0, _p)

import numpy as np

D = 4096          # feature dim (= rotation size)
TOKENS = 8192     # 4 * 2048
N_CORES = 8
T_CORE = TOKENS // N_CORES   # 1024 tokens per core
P = 128           # partitions
HALF = D // 2     # 2048: o/k half size
KH = HALF // P    # 16 contraction chunks per half
OBH = HALF // P   # 16 output blocks per half
TH = T_CORE // 2  # 512 tokens per t-half (= one matmul moving dim)
NPROD = 7
ORDER = (0, 1, 2, 3, 4, 6, 5)  # product emission order (M6 last: 1-stt tail)

_compiled = None


def _matmul_hadU_np(x: np.ndarray) -> np.ndarray:
    """Normalized WHT along the last axis — exact port of the reference
    recursive-butterfly (K == 1 branch), in float64."""
    n = x.shape[-1]
    shape = x.shape
    v = x.reshape(-1, n, 1)
    while v.shape[1] > 1:
        b_, m, c = v.shape
        v = v.reshape(b_, m // 2, 2, c)
        a, b = v[:, :, 0, :], v[:, :, 1, :]
        v = np.concatenate([a + b, a - b], axis=-1)
    return v.reshape(shape) / np.sqrt(n)


def _build_nc():
    import concourse.tile as tile
    from concourse import bacc, mybir

    dt = mybir.dt
    alu = mybir.AluOpType
    nc = bacc.Bacc(None, target_bir_lowering=False)

    xc_d = nc.dram_tensor("xc", [NPROD, P, KH, TH], dt.float16,
                          kind="ExternalInput")
    wc_d = nc.dram_tensor("wc", [NPROD, OBH, P, KH, P], dt.float16,
                          kind="ExternalInput")
    b_d = nc.dram_tensor("bias", [P, 2 * OBH], dt.float32,
                         kind="ExternalInput")
    y_d = nc.dram_tensor("yt", [D, T_CORE], dt.float16, kind="ExternalOutput")

    G = 5     # startup group: o-blocks processed c-outer per product so each
              # arriving x-combo chunk unlocks G matmuls during the DMA ramp
              # (bigger G also lowers the startup DMA demand per PE-second:
              # the three HWDGE queues share only ~300 GB/s aggregate)
    WRING = 8   # W tile ring (4 KB/partition each)
    MRING = 18  # staged-product ring (1 KB/partition each); in-place
                # recombine frees every staged product within 2 positions

    with tile.TileContext(nc) as tc:
        with (
            tc.tile_pool(name="xcp", bufs=1) as xcp,
            tc.tile_pool(name="wp", bufs=WRING) as wp,
            tc.tile_pool(name="mp", bufs=MRING) as mp,
            tc.tile_pool(name="op", bufs=24) as op,
            tc.tile_pool(name="bp", bufs=1) as bp,
            tc.tile_pool(name="pp", bufs=8, space="PSUM") as pp,
        ):
            b_sb = bp.tile([P, 2 * OBH], dt.float32)

            xc_sb = [
                xcp.tile([P, KH, TH], dt.float16, name=f"xc_{j}")
                for j in range(NPROD)
            ]

            # ---- PE clock warmup (HAM ramps over ~3.4us of activity) ----
            dum = bp.tile([P, 256], dt.float16, tag="dum", name="dum")
            nc.vector.memset(dum[:], 0.0)

            w_tiles = {}

            def w_alloc(j, obp):
                t = wp.tile([P, KH, P], dt.float16, tag="w",
                            name=f"w_{j}_{obp}")
                w_tiles[(j, obp)] = t
                return t

            def w_load(j, obp, eng=None):
                t = w_alloc(j, obp)
                (eng or nc.gpsimd).dma_start(t[:], wc_d[j, obp, :, :, :])
                return t

            def xc_load(j, c0, n, eng=None):
                (eng or nc.scalar).dma_start(
                    xc_sb[j][:, c0:c0 + n, :], xc_d[j, :, c0:c0 + n, :])

            # ---- DMA triggers in arrival-need order ----
            # A single HWDGE queue sustains only ~146 GB/s while the startup
            # needs ~300 GB/s (W 145 + x-combos 152), so BOTH streams are
            # spread across all three queues (scalar/sync/gpsimd, ~99 GB/s
            # each) with round-robin assignment.
            QS = (nc.gpsimd, nc.scalar, nc.sync)

            def w_eng(j, obp):
                return QS[(j + obp) % 3]

            def w_load_r(j, obp):
                return w_load(j, obp, eng=w_eng(j, obp))

            def xc_pieces(j, chunks=((0, 5), (5, 5), (10, 6))):
                for i, (c0, n) in enumerate(chunks):
                    xc_load(j, c0, n, eng=QS[(j + i) % 3])

            nc.sync.dma_start(b_sb[:], b_d[:])
            # xc0 + w0 finely chunked for the DMA ramp
            xc_load(0, 0, 1, eng=QS[0])
            xc_load(0, 5, 1, eng=QS[1])
            xc_load(0, 10, 2, eng=QS[2])
            for q in range(2):
                for gob in range(G):
                    t = w_alloc(0, gob) if q == 0 else w_tiles[(0, gob)]
                    w_eng(0, gob).dma_start(
                        t[:, q * 4:(q + 1) * 4, :],
                        wc_d[0, gob, :, q * 4:(q + 1) * 4, :])
            xc_load(0, 1, 2, eng=QS[0])
            xc_load(0, 6, 2, eng=QS[1])
            xc_load(0, 12, 2, eng=QS[2])
            for gob in range(G):
                w_eng(0, gob).dma_start(
                    w_tiles[(0, gob)][:, 8:16, :], wc_d[0, gob, :, 8:16, :])
            xc_load(0, 3, 2, eng=QS[0])
            xc_load(0, 8, 2, eng=QS[1])
            xc_load(0, 14, 2, eng=QS[2])
            xc_pieces(1)
            for gob in range(G):
                w_load_r(1, gob)
            xc_pieces(2)
            for gob in range(G):
                w_load_r(2, gob)
            xc_pieces(3)
            for gob in range(G):
                w_load_r(3, gob)
            xc_pieces(4)
            for gob in range(G):
                w_load_r(4, gob)
            xc_pieces(6)
            for gob in range(G):
                w_load_r(6, gob)
            xc_pieces(5)
            for gob in range(G):
                w_load_r(5, gob)

            # startup W for the first steady block so obp=G starts clean
            for j in ORDER:
                w_load_r(j, G)

            # ---- PE warmup dummies ----
            ps_warm = pp.tile([P, TH], dt.float32, tag="ps", name="ps_w")
            for _ in range(14):
                nc.tensor.matmul(
                    ps_warm[:, 0:256], dum[:, 0:128], dum[:, 0:256],
                    start=True, stop=True,
                )

            stage = {}

            def evict(j, obp, ps):
                m = mp.tile([P, TH], dt.float16, tag="m", name=f"m_{j}_{obp}")
                nc.scalar.copy(m[:], ps[:])
                stage[(j, obp)] = m
                return m

            def product(j, obp, ps=None):
                if ps is None:
                    ps = pp.tile([P, TH], dt.float32, tag="ps",
                                 name=f"ps_{j}_{obp}")
                wt = w_tiles.pop((j, obp))
                for c in range(KH):
                    nc.tensor.matmul(
                        ps[:], wt[:, c, :], xc_sb[j][:, c, :],
                        start=(c == 0), stop=(c == KH - 1),
                    )
                evict(j, obp, ps)

            # Incremental recombine: emit each scalar_tensor_tensor as soon
            # as its staged inputs exist (called with the just-finished j),
            # all on the vector ALU, ACCUMULATING IN PLACE into the output
            # tiles (no temporaries).  Products run in ORDER = (0,1,2,3,4,6,5)
            # so after the LAST product of every o-block only
            # evict -> one stt -> DMA remains.
            #   C11 = M1+M4-M5+M7+bt   C12 = M3+M5+bt
            #   C21 = M2+M4+bb         C22 = M1-M2+M3+M6+bb
            rec = {}

            def recombine_step(obp, j):
                bt = b_sb[:, obp:obp + 1]
                bb = b_sb[:, OBH + obp:OBH + obp + 1]
                m = lambda k: stage[(k, obp)]
                rt = slice(obp * P, (obp + 1) * P)
                rb = slice((OBH + obp) * P, (OBH + obp + 1) * P)
                r = rec.setdefault(obp, {})
                v = nc.vector

                def tl(nm):
                    return op.tile([P, TH], dt.float16, tag="o",
                                   name=f"{nm}_{obp}")

                if j == 1:
                    r["o22"] = tl("o22")
                    v.scalar_tensor_tensor(
                        r["o22"][:], m(0)[:], bb, m(1)[:], alu.add,
                        alu.subtract)
                elif j == 2:
                    v.scalar_tensor_tensor(
                        r["o22"][:], r["o22"][:], 0.0, m(2)[:], alu.add,
                        alu.add)
                elif j == 3:
                    r["o11"] = tl("o11")
                    v.scalar_tensor_tensor(
                        r["o11"][:], m(0)[:], bt, m(3)[:], alu.add, alu.add)
                    o21 = tl("o21")
                    v.scalar_tensor_tensor(
                        o21[:], m(1)[:], bb, m(3)[:], alu.add, alu.add)
                    nc.sync.dma_start(y_d[rb, 0:TH], o21[:])
                elif j == 4:
                    v.scalar_tensor_tensor(
                        r["o11"][:], r["o11"][:], 0.0, m(4)[:], alu.add,
                        alu.subtract)
                    o12 = tl("o12")
                    v.scalar_tensor_tensor(
                        o12[:], m(2)[:], bt, m(4)[:], alu.add, alu.add)
                    nc.sync.dma_start(y_d[rt, TH:T_CORE], o12[:])
                elif j == 6:
                    v.scalar_tensor_tensor(
                        r["o11"][:], r["o11"][:], 0.0, m(6)[:], alu.add,
                        alu.add)
                    nc.sync.dma_start(y_d[rt, 0:TH], r["o11"][:])
                elif j == 5:
                    v.scalar_tensor_tensor(
                        r["o22"][:], r["o22"][:], 0.0, m(5)[:], alu.add,
                        alu.add)
                    nc.sync.dma_start(y_d[rb, TH:T_CORE], r["o22"][:])
                    for k in range(NPROD):
                        del stage[(k, obp)]
                    del rec[obp]

            # ---- startup group: j-major, c-outer across obp 0..G-1 ----
            for j in ORDER:
                ps_j = []
                for gob in range(G):
                    if j == 0 and gob == 0:
                        ps_j.append(ps_warm)
                    else:
                        ps_j.append(pp.tile(
                            [P, TH], dt.float32, tag="ps",
                            name=f"ps_{j}_{gob}"))
                for c in range(KH):
                    for gob in range(G):
                        nc.tensor.matmul(
                            ps_j[gob][:],
                            w_tiles[(j, gob)][:, c, :], xc_sb[j][:, c, :],
                            start=(c == 0), stop=(c == KH - 1),
                        )
                for gob in range(G):
                    evict(j, gob, ps_j[gob])
                for gob in range(G):
                    recombine_step(gob, j)
            for j, gob in list(w_tiles):
                if gob < G:
                    del w_tiles[(j, gob)]

            # ---- steady state: obp-major ----
            for obp in range(G, OBH):
                for j in ORDER:
                    if obp + 1 < OBH:
                        w_load_r(j, obp + 1)
                    product(j, obp)
                    recombine_step(obp, j)

    nc.compile()
    return nc


def _get_nc():
    global _compiled
    if _compiled is None:
        _compiled = _build_nc()
    return _compiled


def _prep_inputs(x, W, b):
    x = np.asarray(x, dtype=np.float32)
    W = np.asarray(W, dtype=np.float32)
    b = np.asarray(b, dtype=np.float32)

    Wr = _matmul_hadU_np(W.astype(np.float64))  # [o, k] float64
    A11 = Wr[:HALF, :HALF]
    A12 = Wr[:HALF, HALF:]
    A21 = Wr[HALF:, :HALF]
    A22 = Wr[HALF:, HALF:]
    WCs = (A11 + A22, A21 + A22, A11, A22, A11 + A12, A21 - A11, A12 - A22)
    # pack[j][obp, p, c, jo] = WC_j[obp*128 + jo, c*128 + p]
    wc = np.stack([
        w.reshape(OBH, P, KH, P).transpose(0, 3, 2, 1) for w in WCs
    ]).astype(np.float16)
    wc = np.ascontiguousarray(wc)

    b_pack = np.ascontiguousarray(b.reshape(2 * OBH, P).T)  # [128, 32]

    xt = x.reshape(N_CORES, T_CORE, D).transpose(0, 2, 1)  # [core, k, t] f32
    B11 = xt[:, :HALF, :TH]
    B12 = xt[:, :HALF, TH:]
    B21 = xt[:, HALF:, :TH]
    B22 = xt[:, HALF:, TH:]
    XCs = (B11 + B22, B11, B12 - B22, B21 - B11, B22, B11 + B12, B21 + B22)
    # pack[core, j, p, c, t] = XC_j[core, c*128 + p, t]
    xc = np.stack([
        c.reshape(N_CORES, KH, P, TH).transpose(0, 2, 1, 3) for c in XCs
    ], axis=1).astype(np.float16)
    xc = np.ascontiguousarray(xc)

    in_maps = [
        {"xc": xc[i], "wc": wc, "bias": b_pack} for i in range(N_CORES)
    ]
    return in_maps


def _assemble(results):
    # yt per core: [4096 o, 1024 t] fp16 -> y[t, o] fp32
    parts = [r["yt"].T.astype(np.float32) for r in results]
    y = np.concatenate(parts, axis=0)  # [8192, 4096]
    return y.reshape(4, 2048, D)


def _run(x, W, b, **spmd_kwargs):
    from concourse.bass_utils import run_bass_kernel_spmd

    nc = _get_nc()
    in_maps = _prep_inputs(x, W, b)
    res = run_bass_kernel_spmd(nc, in_maps, list(range(N_CORES)), **spmd_kwargs)
    return _assemble(res.results), res


def kernel(x, W, b):
    out, _ = _run(x, W, b)
    return out


# revision 20
# speedup vs baseline: 1.0215x; 1.0215x over previous
"""Trainium2 Bass kernel for nn_InputRotationWrapper: y = WHT(x) @ W^T + b.

Algebraic fold: WHT (normalized Walsh-Hadamard along feature dim, H symmetric)
commutes into the weight: y = (x H) W^T = x (W H)^T.  The device runs a pure
GEMM  y = x @ Wr^T + b  with Wr = WHT(W) computed once on the host.

On top of the fold, one level of STRASSEN over 2x2x2 blocking of
(o, k, t) cuts the PE matmul count by 1/8 — the kernel is PE-streaming-bound
at fp16 (1 moving column/cycle), so this is a direct 12.5% win that neither
fp8 (accuracy: e4m3 x,W measures 3.8e-2 rel err vs the 2e-2 gate) nor uint8
(TRN2 silicon zeroes integer matmul products; probed via NEFF dtype patch)
can reach.

  C = Wr @ x^T = [[C11 C12],[C21 C22]],  A = Wr halves, B = x^T halves
  M1=(A11+A22)(B11+B22) M2=(A21+A22)B11 M3=A11(B12-B22) M4=A22(B21-B11)
  M5=(A11+A12)B22 M6=(A21-A11)(B11+B12) M7=(A12-A22)(B21+B22)
  C11=M1+M4-M5+M7  C12=M3+M5  C21=M2+M4  C22=M1-M2+M3+M6

All 7 A-combos (W-side) and 7 B-combos (x-side) are precomputed on the host
in f64/f32 and shipped as fp16: the device only runs products and cheap
recombines.  Per core (1024 tokens, data-parallel over 8 cores):

  - 7 x-combos resident in SBUF: [128p, 16c, 512t] fp16 each (14.7 MB)
  - W-combos streamed per (product j, o-block obp): [128p, 16c, 128o] fp16
  - 16 obp iterations x 7 products x 16-chunk PSUM accumulation
    = 1792 matmuls of 512 cols (vs 2048 classical) ~ 387 us PE wall
  - ScalarE evicts each product PSUM->SBUF fp16; VectorE recombines with
    scalar_tensor_tensor (bias fused via the per-partition scalar operand);
    outputs DMA per [128, 512] slice.  All hidden under PE time.

Startup mirrors the fp16 baseline: PE-clock warmup dummies, then a j-major
group over the first G o-blocks processed c-outer so every arriving x-combo
chunk immediately unlocks G matmuls while the DMA subsystem ramps.
"""
import sys

for _p in ("/opt/trn_rl_repo", "/root/.axon_site/_ro/trn_rl_repo"):
    if _p not in sys.path:
        sys.path.insert(0, _p)

import numpy as np

D = 4096          # feature dim (= rotation size)
TOKENS = 8192     # 4 * 2048
N_CORES = 8
T_CORE = TOKENS // N_CORES   # 1024 tokens per core
P = 128           # partitions
HALF = D // 2     # 2048: o/k half size
KH = HALF // P    # 16 contraction chunks per half
OBH = HALF // P   # 16 output blocks per half
TH = T_CORE // 2  # 512 tokens per t-half (= one matmul moving dim)
NPROD = 7
ORDER = (0, 1, 2, 3, 4, 6, 5)  # product emission order (M6 last: 1-stt tail)

_compiled = None


def _matmul_hadU_np(x: np.ndarray) -> np.ndarray:
    """Normalized WHT along the last axis — exact port of the reference
    recursive-butterfly (K == 1 branch), in float64."""
    n = x.shape[-1]
    shape = x.shape
    v = x.reshape(-1, n, 1)
    while v.shape[1] > 1:
        b_, m, c = v.shape
        v = v.reshape(b_, m // 2, 2, c)
        a, b = v[:, :, 0, :], v[:, :, 1, :]
        v = np.concatenate([a + b, a - b], axis=-1)
    return v.reshape(shape) / np.sqrt(n)


def _build_nc():
    import concourse.tile as tile
    from concourse import bacc, mybir

    dt = mybir.dt
    alu = mybir.AluOpType
    nc = bacc.Bacc(None, target_bir_lowering=False)

    xc_d = nc.dram_tensor("xc", [NPROD, P, KH, TH], dt.float16,
                          kind="ExternalInput")
    wc_d = nc.dram_tensor("wc", [NPROD, OBH, P, KH, P], dt.float16,
                          kind="ExternalInput")
    b_d = nc.dram_tensor("bias", [P, 2 * OBH], dt.float32,
                         kind="ExternalInput")
    y_d = nc.dram_tensor("yt", [D, T_CORE], dt.float16, kind="ExternalOutput")

    G = 5     # startup group: o-blocks processed c-outer per product so each
              # arriving x-combo chunk unlocks G matmuls during the DMA ramp
              # (bigger G also lowers the startup DMA demand per PE-second:
              # the three HWDGE queues share only ~300 GB/s aggregate)
    WRING = 8   # W tile ring (4 KB/partition each)
    MRING = 20  # staged-product ring (1 KB/partition each); in-place
                # recombine frees every staged product within 2 positions

    with tile.TileContext(nc) as tc:
        with (
            tc.tile_pool(name="xcp", bufs=1) as xcp,
            tc.tile_pool(name="wp", bufs=WRING) as wp,
            tc.tile_pool(name="mp", bufs=MRING) as mp,
            tc.tile_pool(name="op", bufs=24) as op,
            tc.tile_pool(name="bp", bufs=1) as bp,
            tc.tile_pool(name="pp", bufs=8, space="PSUM") as pp,
        ):
            b_sb = bp.tile([P, 2 * OBH], dt.float32)

            xc_sb = [
                xcp.tile([P, KH, TH], dt.float16, name=f"xc_{j}")
                for j in range(NPROD)
            ]

            # ---- PE clock warmup (HAM ramps over ~3.4us of activity) ----
            dum = bp.tile([P, 256], dt.float16, tag="dum", name="dum")
            nc.vector.memset(dum[:], 0.0)

            w_tiles = {}

            def w_alloc(j, obp):
                t = wp.tile([P, KH, P], dt.float16, tag="w",
                            name=f"w_{j}_{obp}")
                w_tiles[(j, obp)] = t
                return t

            def w_load(j, obp, eng=None):
                t = w_alloc(j, obp)
                (eng or nc.gpsimd).dma_start(t[:], wc_d[j, obp, :, :, :])
                return t

            def xc_load(j, c0, n, eng=None):
                (eng or nc.scalar).dma_start(
                    xc_sb[j][:, c0:c0 + n, :], xc_d[j, :, c0:c0 + n, :])

            # ---- DMA triggers in arrival-need order ----
            # Queue discipline (one HWDGE queue sustains ~146 GB/s, all
            # three share ~300 GB/s, and a DMA trigger whose ring-WAR isn't
            # met BLOCKS the whole queue behind it):
            #  - x-combos ONLY on scalar+sync (c-halves), emitted before any
            #    W on those queues; their tiles have no ring-WAR, so they
            #    never block.
            #  - W for the first 4 positions on gpsimd alone (~145 GB/s,
            #    marginal but overlapping the cold-clock window); W for the
            #    last 3 positions rides scalar/sync after the x-combos.
            #  - steady W alternates gpsimd/scalar; sync carries outputs.
            nc.sync.dma_start(b_sb[:], b_d[:])
            xc_load(0, 0, 1)
            xc_load(0, 8, 1, eng=nc.sync)
            xc_load(0, 1, 1)
            xc_load(0, 9, 1, eng=nc.sync)
            xc_load(0, 2, 2)
            xc_load(0, 10, 2, eng=nc.sync)
            xc_load(0, 4, 4)
            xc_load(0, 12, 4, eng=nc.sync)
            for j in ORDER[1:]:
                xc_load(j, 0, 8)
                xc_load(j, 8, 8, eng=nc.sync)
            # W: gpsimd carries positions 0..3 (j = 0,1,2,3)
            for q in range(2):
                for gob in range(G):
                    t = w_alloc(0, gob) if q == 0 else w_tiles[(0, gob)]
                    nc.gpsimd.dma_start(
                        t[:, q * 4:(q + 1) * 4, :],
                        wc_d[0, gob, :, q * 4:(q + 1) * 4, :])
            for gob in range(G):
                nc.gpsimd.dma_start(
                    w_tiles[(0, gob)][:, 8:16, :], wc_d[0, gob, :, 8:16, :])
            for j in (1, 2, 3):
                for gob in range(G):
                    w_load(j, gob, eng=nc.gpsimd)
            # positions 4..6 (j = 4, 6, 5) after the x-combos on scalar/sync
            for gob in range(G):
                w_load(4, gob, eng=nc.scalar)
            for gob in range(G):
                w_load(6, gob, eng=nc.sync if gob % 2 else nc.scalar)
            for gob in range(G):
                w_load(5, gob, eng=nc.gpsimd)

            # startup W for the first steady block so obp=G starts clean
            for j in ORDER:
                w_load(j, G, eng=nc.scalar if j % 2 else nc.gpsimd)

            # ---- PE warmup dummies ----
            ps_warm = pp.tile([P, TH], dt.float32, tag="ps", name="ps_w")
            for _ in range(14):
                nc.tensor.matmul(
                    ps_warm[:, 0:256], dum[:, 0:128], dum[:, 0:256],
                    start=True, stop=True,
                )

            stage = {}

            def evict(j, obp, ps):
                m = mp.tile([P, TH], dt.float16, tag="m", name=f"m_{j}_{obp}")
                nc.scalar.copy(m[:], ps[:])
                stage[(j, obp)] = m
                return m

            def product(j, obp, ps=None):
                if ps is None:
                    ps = pp.tile([P, TH], dt.float32, tag="ps",
                                 name=f"ps_{j}_{obp}")
                wt = w_tiles.pop((j, obp))
                for c in range(KH):
                    nc.tensor.matmul(
                        ps[:], wt[:, c, :], xc_sb[j][:, c, :],
                        start=(c == 0), stop=(c == KH - 1),
                    )
                evict(j, obp, ps)

            # Incremental recombine: emit each scalar_tensor_tensor as soon
            # as its staged inputs exist (called with the just-finished j),
            # all on the vector ALU, ACCUMULATING IN PLACE into the output
            # tiles (no temporaries).  Products run in ORDER = (0,1,2,3,4,6,5)
            # so after the LAST product of every o-block only
            # evict -> one stt -> DMA remains.
            #   C11 = M1+M4-M5+M7+bt   C12 = M3+M5+bt
            #   C21 = M2+M4+bb         C22 = M1-M2+M3+M6+bb
            rec = {}

            def recombine_step(obp, j):
                bt = b_sb[:, obp:obp + 1]
                bb = b_sb[:, OBH + obp:OBH + obp + 1]
                m = lambda k: stage[(k, obp)]
                rt = slice(obp * P, (obp + 1) * P)
                rb = slice((OBH + obp) * P, (OBH + obp + 1) * P)
                r = rec.setdefault(obp, {})
                v = nc.vector

                def tl(nm):
                    return op.tile([P, TH], dt.float16, tag="o",
                                   name=f"{nm}_{obp}")

                if j == 1:
                    r["o22"] = tl("o22")
                    v.scalar_tensor_tensor(
                        r["o22"][:], m(0)[:], bb, m(1)[:], alu.add,
                        alu.subtract)
                elif j == 2:
                    v.scalar_tensor_tensor(
                        r["o22"][:], r["o22"][:], 0.0, m(2)[:], alu.add,
                        alu.add)
                elif j == 3:
                    r["o11"] = tl("o11")
                    v.scalar_tensor_tensor(
                        r["o11"][:], m(0)[:], bt, m(3)[:], alu.add, alu.add)
                    o21 = tl("o21")
                    v.scalar_tensor_tensor(
                        o21[:], m(1)[:], bb, m(3)[:], alu.add, alu.add)
                    nc.sync.dma_start(y_d[rb, 0:TH], o21[:])
                elif j == 4:
                    v.scalar_tensor_tensor(
                        r["o11"][:], r["o11"][:], 0.0, m(4)[:], alu.add,
                        alu.subtract)
                    o12 = tl("o12")
                    v.scalar_tensor_tensor(
                        o12[:], m(2)[:], bt, m(4)[:], alu.add, alu.add)
                    nc.sync.dma_start(y_d[rt, TH:T_CORE], o12[:])
                elif j == 6:
                    v.scalar_tensor_tensor(
                        r["o11"][:], r["o11"][:], 0.0, m(6)[:], alu.add,
                        alu.add)
                    nc.sync.dma_start(y_d[rt, 0:TH], r["o11"][:])
                elif j == 5:
                    v.scalar_tensor_tensor(
                        r["o22"][:], r["o22"][:], 0.0, m(5)[:], alu.add,
                        alu.add)
                    nc.sync.dma_start(y_d[rb, TH:T_CORE], r["o22"][:])
                    for k in range(NPROD):
                        del stage[(k, obp)]
                    del rec[obp]

            # ---- startup group: j-major, c-outer across obp 0..G-1 ----
            for j in ORDER:
                ps_j = []
                for gob in range(G):
                    if j == 0 and gob == 0:
                        ps_j.append(ps_warm)
                    else:
                        ps_j.append(pp.tile(
                            [P, TH], dt.float32, tag="ps",
                            name=f"ps_{j}_{gob}"))
                for c in range(KH):
                    for gob in range(G):
                        nc.tensor.matmul(
                            ps_j[gob][:],
                            w_tiles[(j, gob)][:, c, :], xc_sb[j][:, c, :],
                            start=(c == 0), stop=(c == KH - 1),
                        )
                for gob in range(G):
                    evict(j, gob, ps_j[gob])
                for gob in range(G):
                    recombine_step(gob, j)
            for j, gob in list(w_tiles):
                if gob < G:
                    del w_tiles[(j, gob)]

            # ---- steady state: obp-major ----
            for obp in range(G, OBH):
                for j in ORDER:
                    if obp + 1 < OBH:
                        w_load(j, obp + 1,
                               eng=nc.scalar if (j + obp) % 2 else nc.gpsimd)
                    product(j, obp)
                    recombine_step(obp, j)

    nc.compile()
    return nc


def _get_nc():
    global _compiled
    if _compiled is None:
        _compiled = _build_nc()
    return _compiled


def _prep_inputs(x, W, b):
    x = np.asarray(x, dtype=np.float32)
    W = np.asarray(W, dtype=np.float32)
    b = np.asarray(b, dtype=np.float32)

    Wr = _matmul_hadU_np(W.astype(np.float64))  # [o, k] float64
    A11 = Wr[:HALF, :HALF]
    A12 = Wr[:HALF, HALF:]
    A21 = Wr[HALF:, :HALF]
    A22 = Wr[HALF:, HALF:]
    WCs = (A11 + A22, A21 + A22, A11, A22, A11 + A12, A21 - A11, A12 - A22)
    # pack[j][obp, p, c, jo] = WC_j[obp*128 + jo, c*128 + p]
    wc = np.stack([
        w.reshape(OBH, P, KH, P).transpose(0, 3, 2, 1) for w in WCs
    ]).astype(np.float16)
    wc = np.ascontiguousarray(wc)

    b_pack = np.ascontiguousarray(b.reshape(2 * OBH, P).T)  # [128, 32]

    xt = x.reshape(N_CORES, T_CORE, D).transpose(0, 2, 1)  # [core, k, t] f32
    B11 = xt[:, :HALF, :TH]
    B12 = xt[:, :HALF, TH:]
    B21 = xt[:, HALF:, :TH]
    B22 = xt[:, HALF:, TH:]
    XCs = (B11 + B22, B11, B12 - B22, B21 - B11, B22, B11 + B12, B21 + B22)
    # pack[core, j, p, c, t] = XC_j[core, c*128 + p, t]
    xc = np.stack([
        c.reshape(N_CORES, KH, P, TH).transpose(0, 2, 1, 3) for c in XCs
    ], axis=1).astype(np.float16)
    xc = np.ascontiguousarray(xc)

    in_maps = [
        {"xc": xc[i], "wc": wc, "bias": b_pack} for i in range(N_CORES)
    ]
    return in_maps


def _assemble(results):
    # yt per core: [4096 o, 1024 t] fp16 -> y[t, o] fp32
    parts = [r["yt"].T.astype(np.float32) for r in results]
    y = np.concatenate(parts, axis=0)  # [8192, 4096]
    return y.reshape(4, 2048, D)


def _run(x, W, b, **spmd_kwargs):
    from concourse.bass_utils import run_bass_kernel_spmd

    nc = _get_nc()
    in_maps = _prep_inputs(x, W, b)
    res = run_bass_kernel_spmd(nc, in_maps, list(range(N_CORES)), **spmd_kwargs)
    return _assemble(res.results), res


def kernel(x, W, b):
    out, _ = _run(x, W, b)
    return out


# revision 21
# speedup vs baseline: 1.0404x; 1.0185x over previous
"""Trainium2 Bass kernel for nn_InputRotationWrapper: y = WHT(x) @ W^T + b.

Algebraic fold: WHT (normalized Walsh-Hadamard along feature dim, H symmetric)
commutes into the weight: y = (x H) W^T = x (W H)^T.  The device runs a pure
GEMM  y = x @ Wr^T + b  with Wr = WHT(W) computed once on the host.

On top of the fold, one level of STRASSEN over 2x2x2 blocking of
(o, k, t) cuts the PE matmul count by 1/8 — the kernel is PE-streaming-bound
at fp16 (1 moving column/cycle), so this is a direct 12.5% win that neither
fp8 (accuracy: e4m3 x,W measures 3.8e-2 rel err vs the 2e-2 gate) nor uint8
(TRN2 silicon zeroes integer matmul products; probed via NEFF dtype patch)
can reach.

  C = Wr @ x^T = [[C11 C12],[C21 C22]],  A = Wr halves, B = x^T halves
  M1=(A11+A22)(B11+B22) M2=(A21+A22)B11 M3=A11(B12-B22) M4=A22(B21-B11)
  M5=(A11+A12)B22 M6=(A21-A11)(B11+B12) M7=(A12-A22)(B21+B22)
  C11=M1+M4-M5+M7  C12=M3+M5  C21=M2+M4  C22=M1-M2+M3+M6

All 7 A-combos (W-side) and 7 B-combos (x-side) are precomputed on the host
in f64/f32 and shipped as fp16: the device only runs products and cheap
recombines.  Per core (1024 tokens, data-parallel over 8 cores):

  - 7 x-combos resident in SBUF: [128p, 16c, 512t] fp16 each (14.7 MB)
  - W-combos streamed per (product j, o-block obp): [128p, 16c, 128o] fp16
  - 16 obp iterations x 7 products x 16-chunk PSUM accumulation
    = 1792 matmuls of 512 cols (vs 2048 classical) ~ 387 us PE wall
  - ScalarE evicts each product PSUM->SBUF fp16; VectorE recombines with
    scalar_tensor_tensor (bias fused via the per-partition scalar operand);
    outputs DMA per [128, 512] slice.  All hidden under PE time.

Startup mirrors the fp16 baseline: PE-clock warmup dummies, then a j-major
group over the first G o-blocks processed c-outer so every arriving x-combo
chunk immediately unlocks G matmuls while the DMA subsystem ramps.
"""
import sys

for _p in ("/opt/trn_rl_repo", "/root/.axon_site/_ro/trn_rl_repo"):
    if _p not in sys.path:
        sys.path.insert(0, _p)

import numpy as np

D = 4096          # feature dim (= rotation size)
TOKENS = 8192     # 4 * 2048
N_CORES = 8
T_CORE = TOKENS // N_CORES   # 1024 tokens per core
P = 128           # partitions
HALF = D // 2     # 2048: o/k half size
KH = HALF // P    # 16 contraction chunks per half
OBH = HALF // P   # 16 output blocks per half
TH = T_CORE // 2  # 512 tokens per t-half (= one matmul moving dim)
NPROD = 7
ORDER = (0, 1, 2, 3, 4, 6, 5)  # product emission order (M6 last: 1-stt tail)

_compiled = None


def _matmul_hadU_np(x: np.ndarray) -> np.ndarray:
    """Normalized WHT along the last axis — exact port of the reference
    recursive-butterfly (K == 1 branch), in float64."""
    n = x.shape[-1]
    shape = x.shape
    v = x.reshape(-1, n, 1)
    while v.shape[1] > 1:
        b_, m, c = v.shape
        v = v.reshape(b_, m // 2, 2, c)
        a, b = v[:, :, 0, :], v[:, :, 1, :]
        v = np.concatenate([a + b, a - b], axis=-1)
    return v.reshape(shape) / np.sqrt(n)


def _build_nc():
    import concourse.tile as tile
    from concourse import bacc, mybir

    dt = mybir.dt
    alu = mybir.AluOpType
    nc = bacc.Bacc(None, target_bir_lowering=False)

    xc_d = nc.dram_tensor("xc", [NPROD, P, KH, TH], dt.float16,
                          kind="ExternalInput")
    wc_d = nc.dram_tensor("wc", [NPROD, OBH, P, KH, P], dt.float16,
                          kind="ExternalInput")
    b_d = nc.dram_tensor("bias", [P, 2 * OBH], dt.float32,
                         kind="ExternalInput")
    y_d = nc.dram_tensor("yt", [D, T_CORE], dt.float16, kind="ExternalOutput")

    G = 5     # startup group: o-blocks processed c-outer per product so each
              # arriving x-combo chunk unlocks G matmuls during the DMA ramp
              # (bigger G also lowers the startup DMA demand per PE-second:
              # the three HWDGE queues share only ~300 GB/s aggregate)
    WRING = 8   # W tile ring (4 KB/partition each)
    MRING = 20  # staged-product ring (1 KB/partition each); in-place
                # recombine frees every staged product within 2 positions

    with tile.TileContext(nc) as tc:
        with (
            tc.tile_pool(name="xcp", bufs=1) as xcp,
            tc.tile_pool(name="wp", bufs=WRING) as wp,
            tc.tile_pool(name="mp", bufs=MRING) as mp,
            tc.tile_pool(name="op", bufs=24) as op,
            tc.tile_pool(name="bp", bufs=1) as bp,
            tc.tile_pool(name="pp", bufs=8, space="PSUM") as pp,
        ):
            b_sb = bp.tile([P, 2 * OBH], dt.float32)

            xc_sb = [
                xcp.tile([P, KH, TH], dt.float16, name=f"xc_{j}")
                for j in range(NPROD)
            ]

            # ---- PE clock warmup (HAM ramps over ~3.4us of activity) ----
            dum = bp.tile([P, 256], dt.float16, tag="dum", name="dum")
            nc.vector.memset(dum[:], 0.0)

            w_tiles = {}

            def w_alloc(j, obp):
                t = wp.tile([P, KH, P], dt.float16, tag="w",
                            name=f"w_{j}_{obp}")
                w_tiles[(j, obp)] = t
                return t

            def w_load(j, obp, eng=None):
                t = w_alloc(j, obp)
                (eng or nc.gpsimd).dma_start(t[:], wc_d[j, obp, :, :, :])
                return t

            def xc_load(j, c0, n, eng=None):
                (eng or nc.scalar).dma_start(
                    xc_sb[j][:, c0:c0 + n, :], xc_d[j, :, c0:c0 + n, :])

            # ---- DMA triggers in arrival-need order ----
            # Queue discipline (one HWDGE queue sustains ~146 GB/s, all
            # three share ~300 GB/s, and a DMA trigger whose ring-WAR isn't
            # met BLOCKS the whole queue behind it):
            #  - x-combos ONLY on scalar+sync (c-halves), emitted before any
            #    W on those queues; their tiles have no ring-WAR, so they
            #    never block.
            #  - W for the first 4 positions on gpsimd alone (~145 GB/s,
            #    marginal but overlapping the cold-clock window); W for the
            #    last 3 positions rides scalar/sync after the x-combos.
            #  - steady W alternates gpsimd/scalar; sync carries outputs.
            nc.sync.dma_start(b_sb[:], b_d[:])
            QR = (nc.gpsimd, nc.scalar, nc.sync)
            # pos0: xc0 chunks + w0 quarters interleaved over all 3 queues
            xc_load(0, 0, 1)
            xc_load(0, 8, 1, eng=nc.sync)
            for gob in range(G):
                t = w_alloc(0, gob)
                QR[gob % 3].dma_start(t[:, 0:4, :], wc_d[0, gob, :, 0:4, :])
            xc_load(0, 1, 1)
            xc_load(0, 9, 1, eng=nc.sync)
            for gob in range(G):
                QR[(gob + 1) % 3].dma_start(
                    w_tiles[(0, gob)][:, 4:8, :], wc_d[0, gob, :, 4:8, :])
            xc_load(0, 2, 2)
            xc_load(0, 10, 2, eng=nc.sync)
            for gob in range(G):
                QR[(gob + 2) % 3].dma_start(
                    w_tiles[(0, gob)][:, 8:16, :], wc_d[0, gob, :, 8:16, :])
            xc_load(0, 4, 4)
            xc_load(0, 12, 4, eng=nc.sync)
            # pos1 W early (gpsimd), then remaining x-combos, then later W
            for gob in range(G):
                w_load(1, gob, eng=nc.gpsimd)
            for j in ORDER[1:]:
                xc_load(j, 0, 8)
                xc_load(j, 8, 8, eng=nc.sync)
            for j in (2, 3):
                for gob in range(G):
                    w_load(j, gob, eng=nc.gpsimd)
            # positions 4..6 (j = 4, 6, 5) after the x-combos on scalar/sync
            for gob in range(G):
                w_load(4, gob, eng=nc.scalar)
            for gob in range(G):
                w_load(6, gob, eng=nc.sync if gob % 2 else nc.scalar)
            for gob in range(G):
                w_load(5, gob, eng=nc.gpsimd)

            # startup W for the first steady block so obp=G starts clean
            for j in ORDER:
                w_load(j, G, eng=nc.scalar if j % 2 else nc.gpsimd)

            # ---- PE warmup dummies ----
            # ~95 x 107ns(cold) of dummy matmuls: keeps the PE busy (and the
            # HAM clock un-throttled) through the whole DMA ramp instead of
            # idling in data stalls that re-throttle the clock each time.
            ps_warm = pp.tile([P, TH], dt.float32, tag="ps", name="ps_w")
            for _ in range(95):
                nc.tensor.matmul(
                    ps_warm[:, 0:256], dum[:, 0:128], dum[:, 0:256],
                    start=True, stop=True,
                )

            stage = {}

            def evict(j, obp, ps):
                m = mp.tile([P, TH], dt.float16, tag="m", name=f"m_{j}_{obp}")
                nc.scalar.copy(m[:], ps[:])
                stage[(j, obp)] = m
                return m

            def product(j, obp, ps=None):
                if ps is None:
                    ps = pp.tile([P, TH], dt.float32, tag="ps",
                                 name=f"ps_{j}_{obp}")
                wt = w_tiles.pop((j, obp))
                for c in range(KH):
                    nc.tensor.matmul(
                        ps[:], wt[:, c, :], xc_sb[j][:, c, :],
                        start=(c == 0), stop=(c == KH - 1),
                    )
                evict(j, obp, ps)

            # Incremental recombine: emit each scalar_tensor_tensor as soon
            # as its staged inputs exist (called with the just-finished j),
            # all on the vector ALU, ACCUMULATING IN PLACE into the output
            # tiles (no temporaries).  Products run in ORDER = (0,1,2,3,4,6,5)
            # so after the LAST product of every o-block only
            # evict -> one stt -> DMA remains.
            #   C11 = M1+M4-M5+M7+bt   C12 = M3+M5+bt
            #   C21 = M2+M4+bb         C22 = M1-M2+M3+M6+bb
            rec = {}

            def recombine_step(obp, j):
                bt = b_sb[:, obp:obp + 1]
                bb = b_sb[:, OBH + obp:OBH + obp + 1]
                m = lambda k: stage[(k, obp)]
                rt = slice(obp * P, (obp + 1) * P)
                rb = slice((OBH + obp) * P, (OBH + obp + 1) * P)
                r = rec.setdefault(obp, {})
                v = nc.vector

                def tl(nm):
                    return op.tile([P, TH], dt.float16, tag="o",
                                   name=f"{nm}_{obp}")

                if j == 1:
                    r["o22"] = tl("o22")
                    v.scalar_tensor_tensor(
                        r["o22"][:], m(0)[:], bb, m(1)[:], alu.add,
                        alu.subtract)
                elif j == 2:
                    v.scalar_tensor_tensor(
                        r["o22"][:], r["o22"][:], 0.0, m(2)[:], alu.add,
                        alu.add)
                elif j == 3:
                    r["o11"] = tl("o11")
                    v.scalar_tensor_tensor(
                        r["o11"][:], m(0)[:], bt, m(3)[:], alu.add, alu.add)
                    o21 = tl("o21")
                    v.scalar_tensor_tensor(
                        o21[:], m(1)[:], bb, m(3)[:], alu.add, alu.add)
                    nc.sync.dma_start(y_d[rb, 0:TH], o21[:])
                elif j == 4:
                    v.scalar_tensor_tensor(
                        r["o11"][:], r["o11"][:], 0.0, m(4)[:], alu.add,
                        alu.subtract)
                    o12 = tl("o12")
                    v.scalar_tensor_tensor(
                        o12[:], m(2)[:], bt, m(4)[:], alu.add, alu.add)
                    nc.sync.dma_start(y_d[rt, TH:T_CORE], o12[:])
                elif j == 6:
                    v.scalar_tensor_tensor(
                        r["o11"][:], r["o11"][:], 0.0, m(6)[:], alu.add,
                        alu.add)
                    nc.sync.dma_start(y_d[rt, 0:TH], r["o11"][:])
                elif j == 5:
                    v.scalar_tensor_tensor(
                        r["o22"][:], r["o22"][:], 0.0, m(5)[:], alu.add,
                        alu.add)
                    nc.sync.dma_start(y_d[rb, TH:T_CORE], r["o22"][:])
                    for k in range(NPROD):
                        del stage[(k, obp)]
                    del rec[obp]

            # ---- startup group: j-major, c-outer across obp 0..G-1 ----
            for j in ORDER:
                ps_j = []
                for gob in range(G):
                    if j == 0 and gob == 0:
                        ps_j.append(ps_warm)
                    else:
                        ps_j.append(pp.tile(
                            [P, TH], dt.float32, tag="ps",
                            name=f"ps_{j}_{gob}"))
                for c in range(KH):
                    for gob in range(G):
                        nc.tensor.matmul(
                            ps_j[gob][:],
                            w_tiles[(j, gob)][:, c, :], xc_sb[j][:, c, :],
                            start=(c == 0), stop=(c == KH - 1),
                        )
                for gob in range(G):
                    evict(j, gob, ps_j[gob])
                for gob in range(G):
                    recombine_step(gob, j)
            for j, gob in list(w_tiles):
                if gob < G:
                    del w_tiles[(j, gob)]

            # ---- steady state: obp-major ----
            for obp in range(G, OBH):
                for j in ORDER:
                    if obp + 1 < OBH:
                        w_load(j, obp + 1,
                               eng=nc.scalar if (j + obp) % 2 else nc.gpsimd)
                    product(j, obp)
                    recombine_step(obp, j)

    nc.compile()
    return nc


def _get_nc():
    global _compiled
    if _compiled is None:
        _compiled = _build_nc()
    return _compiled


def _prep_inputs(x, W, b):
    x = np.asarray(x, dtype=np.float32)
    W = np.asarray(W, dtype=np.float32)
    b = np.asarray(b, dtype=np.float32)

    Wr = _matmul_hadU_np(W.astype(np.float64))  # [o, k] float64
    A11 = Wr[:HALF, :HALF]
    A12 = Wr[:HALF, HALF:]
    A21 = Wr[HALF:, :HALF]
    A22 = Wr[HALF:, HALF:]
    WCs = (A11 + A22, A21 + A22, A11, A22, A11 + A12, A21 - A11, A12 - A22)
    # pack[j][obp, p, c, jo] = WC_j[obp*128 + jo, c*128 + p]
    wc = np.stack([
        w.reshape(OBH, P, KH, P).transpose(0, 3, 2, 1) for w in WCs
    ]).astype(np.float16)
    wc = np.ascontiguousarray(wc)

    b_pack = np.ascontiguousarray(b.reshape(2 * OBH, P).T)  # [128, 32]

    xt = x.reshape(N_CORES, T_CORE, D).transpose(0, 2, 1)  # [core, k, t] f32
    B11 = xt[:, :HALF, :TH]
    B12 = xt[:, :HALF, TH:]
    B21 = xt[:, HALF:, :TH]
    B22 = xt[:, HALF:, TH:]
    XCs = (B11 + B22, B11, B12 - B22, B21 - B11, B22, B11 + B12, B21 + B22)
    # pack[core, j, p, c, t] = XC_j[core, c*128 + p, t]
    xc = np.stack([
        c.reshape(N_CORES, KH, P, TH).transpose(0, 2, 1, 3) for c in XCs
    ], axis=1).astype(np.float16)
    xc = np.ascontiguousarray(xc)

    in_maps = [
        {"xc": xc[i], "wc": wc, "bias": b_pack} for i in range(N_CORES)
    ]
    return in_maps


def _assemble(results):
    # yt per core: [4096 o, 1024 t] fp16 -> y[t, o] fp32
    parts = [r["yt"].T.astype(np.float32) for r in results]
    y = np.concatenate(parts, axis=0)  # [8192, 4096]
    return y.reshape(4, 2048, D)


def _run(x, W, b, **spmd_kwargs):
    from concourse.bass_utils import run_bass_kernel_spmd

    nc = _get_nc()
    in_maps = _prep_inputs(x, W, b)
    res = run_bass_kernel_spmd(nc, in_maps, list(range(N_CORES)), **spmd_kwargs)
    return _assemble(res.results), res


def kernel(x, W, b):
    out, _ = _run(x, W, b)
    return out


# revision 22
# speedup vs baseline: 1.0565x; 1.0154x over previous
"""Trainium2 Bass kernel for nn_InputRotationWrapper: y = WHT(x) @ W^T + b.

Algebraic fold: WHT (normalized Walsh-Hadamard along feature dim, H symmetric)
commutes into the weight: y = (x H) W^T = x (W H)^T.  The device runs a pure
GEMM  y = x @ Wr^T + b  with Wr = WHT(W) computed once on the host.

On top of the fold, one level of STRASSEN over 2x2x2 blocking of
(o, k, t) cuts the PE matmul count by 1/8 — the kernel is PE-streaming-bound
at fp16 (1 moving column/cycle), so this is a direct 12.5% win that neither
fp8 (accuracy: e4m3 x,W measures 3.8e-2 rel err vs the 2e-2 gate) nor uint8
(TRN2 silicon zeroes integer matmul products; probed via NEFF dtype patch)
can reach.

  C = Wr @ x^T = [[C11 C12],[C21 C22]],  A = Wr halves, B = x^T halves
  M1=(A11+A22)(B11+B22) M2=(A21+A22)B11 M3=A11(B12-B22) M4=A22(B21-B11)
  M5=(A11+A12)B22 M6=(A21-A11)(B11+B12) M7=(A12-A22)(B21+B22)
  C11=M1+M4-M5+M7  C12=M3+M5  C21=M2+M4  C22=M1-M2+M3+M6

All 7 A-combos (W-side) and 7 B-combos (x-side) are precomputed on the host
in f64/f32 and shipped as fp16: the device only runs products and cheap
recombines.  Per core (1024 tokens, data-parallel over 8 cores):

  - 7 x-combos resident in SBUF: [128p, 16c, 512t] fp16 each (14.7 MB)
  - W-combos streamed per (product j, o-block obp): [128p, 16c, 128o] fp16
  - 16 obp iterations x 7 products x 16-chunk PSUM accumulation
    = 1792 matmuls of 512 cols (vs 2048 classical) ~ 387 us PE wall
  - ScalarE evicts each product PSUM->SBUF fp16; VectorE recombines with
    scalar_tensor_tensor (bias fused via the per-partition scalar operand);
    outputs DMA per [128, 512] slice.  All hidden under PE time.

Startup mirrors the fp16 baseline: PE-clock warmup dummies, then a j-major
group over the first G o-blocks processed c-outer so every arriving x-combo
chunk immediately unlocks G matmuls while the DMA subsystem ramps.
"""
import sys

for _p in ("/opt/trn_rl_repo", "/root/.axon_site/_ro/trn_rl_repo"):
    if _p not in sys.path:
        sys.path.insert(0, _p)

import numpy as np

D = 4096          # feature dim (= rotation size)
TOKENS = 8192     # 4 * 2048
N_CORES = 8
T_CORE = TOKENS // N_CORES   # 1024 tokens per core
P = 128           # partitions
HALF = D // 2     # 2048: o/k half size
KH = HALF // P    # 16 contraction chunks per half
OBH = HALF // P   # 16 output blocks per half
TH = T_CORE // 2  # 512 tokens per t-half (= one matmul moving dim)
NPROD = 7
ORDER = (0, 1, 2, 3, 4, 6, 5)  # product emission order (M6 last: 1-stt tail)

_compiled = None


def _matmul_hadU_np(x: np.ndarray) -> np.ndarray:
    """Normalized WHT along the last axis — exact port of the reference
    recursive-butterfly (K == 1 branch), in float64."""
    n = x.shape[-1]
    shape = x.shape
    v = x.reshape(-1, n, 1)
    while v.shape[1] > 1:
        b_, m, c = v.shape
        v = v.reshape(b_, m // 2, 2, c)
        a, b = v[:, :, 0, :], v[:, :, 1, :]
        v = np.concatenate([a + b, a - b], axis=-1)
    return v.reshape(shape) / np.sqrt(n)


def _build_nc():
    import concourse.tile as tile
    from concourse import bacc, mybir

    dt = mybir.dt
    alu = mybir.AluOpType
    nc = bacc.Bacc(None, target_bir_lowering=False)

    xc_d = nc.dram_tensor("xc", [NPROD, P, KH, TH], dt.float16,
                          kind="ExternalInput")
    wc_d = nc.dram_tensor("wc", [NPROD, OBH, P, KH, P], dt.float16,
                          kind="ExternalInput")
    b_d = nc.dram_tensor("bias", [P, 2 * OBH], dt.float32,
                         kind="ExternalInput")
    y_d = nc.dram_tensor("yt", [D, T_CORE], dt.float16, kind="ExternalOutput")

    G = 4     # startup group: o-blocks processed c-outer per product so each
              # arriving x-combo chunk unlocks G matmuls during the DMA ramp.
              # G=4 keeps 2 full positions inside WRING=8: a larger G makes
              # next-position W triggers ring-block behind current compute.
    WRING = 8   # W tile ring (4 KB/partition each)
    MRING = 20  # staged-product ring (1 KB/partition each); in-place
                # recombine frees every staged product within 2 positions

    with tile.TileContext(nc) as tc:
        with (
            tc.tile_pool(name="xcp", bufs=1) as xcp,
            tc.tile_pool(name="wp", bufs=WRING) as wp,
            tc.tile_pool(name="mp", bufs=MRING) as mp,
            tc.tile_pool(name="op", bufs=24) as op,
            tc.tile_pool(name="bp", bufs=1) as bp,
            tc.tile_pool(name="pp", bufs=8, space="PSUM") as pp,
        ):
            b_sb = bp.tile([P, 2 * OBH], dt.float32)

            xc_sb = [
                xcp.tile([P, KH, TH], dt.float16, name=f"xc_{j}")
                for j in range(NPROD)
            ]

            # ---- PE clock warmup (HAM ramps over ~3.4us of activity) ----
            dum = bp.tile([P, 256], dt.float16, tag="dum", name="dum")
            nc.vector.memset(dum[:], 0.0)

            w_tiles = {}

            def w_alloc(j, obp):
                t = wp.tile([P, KH, P], dt.float16, tag="w",
                            name=f"w_{j}_{obp}")
                w_tiles[(j, obp)] = t
                return t

            def w_load(j, obp, eng=None):
                t = w_alloc(j, obp)
                (eng or nc.gpsimd).dma_start(t[:], wc_d[j, obp, :, :, :])
                return t

            def xc_load(j, c0, n, eng=None):
                (eng or nc.scalar).dma_start(
                    xc_sb[j][:, c0:c0 + n, :], xc_d[j, :, c0:c0 + n, :])

            # ---- DMA triggers in arrival-need order ----
            # One HWDGE queue sustains ~146 GB/s and the three together
            # ~300 GB/s; a DMA trigger whose ring-WAR isn't met blocks its
            # whole queue.  Startup demand is ~300 GB/s, so: W alternates
            # gpsimd (even positions) / scalar (odd positions), x-combos go
            # as c-halves on scalar/sync ordered BEFORE the same-position W,
            # and sync also carries bias + outputs.
            nc.sync.dma_start(b_sb[:], b_d[:])
            # pos0: xc0 + w0, finely chunked for the DMA ramp
            xc_load(0, 0, 1)
            xc_load(0, 8, 2, eng=nc.sync)
            for gob in range(G):
                t = w_alloc(0, gob)
                nc.gpsimd.dma_start(t[:, 0:4, :], wc_d[0, gob, :, 0:4, :])
            xc_load(0, 1, 1)
            xc_load(0, 10, 2, eng=nc.sync)
            for gob in range(G):
                nc.gpsimd.dma_start(
                    w_tiles[(0, gob)][:, 4:8, :], wc_d[0, gob, :, 4:8, :])
            xc_load(0, 2, 2)
            xc_load(0, 12, 4, eng=nc.sync)
            for gob in range(G):
                nc.gpsimd.dma_start(
                    w_tiles[(0, gob)][:, 8:16, :], wc_d[0, gob, :, 8:16, :])
            xc_load(0, 4, 4)
            # pos1..6: per position, x-combo halves then W (W queue
            # alternates so each queue carries every other position)
            xc_load(1, 0, 8)
            xc_load(1, 8, 8, eng=nc.sync)
            for gob in range(G):
                w_load(1, gob, eng=nc.scalar)
            xc_load(2, 0, 8)
            xc_load(2, 8, 8, eng=nc.sync)
            for gob in range(G):
                w_load(2, gob, eng=nc.gpsimd)
            xc_load(3, 0, 8)
            xc_load(3, 8, 8, eng=nc.sync)
            for gob in range(G):
                w_load(3, gob, eng=nc.scalar)
            xc_load(4, 0, 8)
            xc_load(4, 8, 8, eng=nc.sync)
            for gob in range(G):
                w_load(4, gob, eng=nc.gpsimd)
            xc_load(6, 0, 8, eng=nc.sync)
            xc_load(6, 8, 8, eng=nc.sync)
            for gob in range(G):
                w_load(6, gob, eng=nc.scalar)
            xc_load(5, 0, 8, eng=nc.sync)
            xc_load(5, 8, 8, eng=nc.sync)
            for gob in range(G):
                w_load(5, gob, eng=nc.gpsimd)

            # startup W for the first steady block so obp=G starts clean
            for j in ORDER:
                w_load(j, G, eng=nc.scalar if j % 2 else nc.gpsimd)

            # ---- PE warmup dummies ----
            # ~95 x 107ns(cold) of dummy matmuls: keeps the PE busy (and the
            # HAM clock un-throttled) through the whole DMA ramp instead of
            # idling in data stalls that re-throttle the clock each time.
            ps_warm = pp.tile([P, TH], dt.float32, tag="ps", name="ps_w")
            for _ in range(80):
                nc.tensor.matmul(
                    ps_warm[:, 0:256], dum[:, 0:128], dum[:, 0:256],
                    start=True, stop=True,
                )

            stage = {}

            def evict(j, obp, ps):
                m = mp.tile([P, TH], dt.float16, tag="m", name=f"m_{j}_{obp}")
                nc.scalar.copy(m[:], ps[:])
                stage[(j, obp)] = m
                return m

            def product(j, obp, ps=None):
                if ps is None:
                    ps = pp.tile([P, TH], dt.float32, tag="ps",
                                 name=f"ps_{j}_{obp}")
                wt = w_tiles.pop((j, obp))
                for c in range(KH):
                    nc.tensor.matmul(
                        ps[:], wt[:, c, :], xc_sb[j][:, c, :],
                        start=(c == 0), stop=(c == KH - 1),
                    )
                evict(j, obp, ps)

            # Incremental recombine: emit each scalar_tensor_tensor as soon
            # as its staged inputs exist (called with the just-finished j),
            # all on the vector ALU, ACCUMULATING IN PLACE into the output
            # tiles (no temporaries).  Products run in ORDER = (0,1,2,3,4,6,5)
            # so after the LAST product of every o-block only
            # evict -> one stt -> DMA remains.
            #   C11 = M1+M4-M5+M7+bt   C12 = M3+M5+bt
            #   C21 = M2+M4+bb         C22 = M1-M2+M3+M6+bb
            rec = {}

            def recombine_step(obp, j):
                bt = b_sb[:, obp:obp + 1]
                bb = b_sb[:, OBH + obp:OBH + obp + 1]
                m = lambda k: stage[(k, obp)]
                rt = slice(obp * P, (obp + 1) * P)
                rb = slice((OBH + obp) * P, (OBH + obp + 1) * P)
                r = rec.setdefault(obp, {})
                v = nc.vector

                def tl(nm):
                    return op.tile([P, TH], dt.float16, tag="o",
                                   name=f"{nm}_{obp}")

                if j == 1:
                    r["o22"] = tl("o22")
                    v.scalar_tensor_tensor(
                        r["o22"][:], m(0)[:], bb, m(1)[:], alu.add,
                        alu.subtract)
                elif j == 2:
                    v.scalar_tensor_tensor(
                        r["o22"][:], r["o22"][:], 0.0, m(2)[:], alu.add,
                        alu.add)
                elif j == 3:
                    r["o11"] = tl("o11")
                    v.scalar_tensor_tensor(
                        r["o11"][:], m(0)[:], bt, m(3)[:], alu.add, alu.add)
                    o21 = tl("o21")
                    v.scalar_tensor_tensor(
                        o21[:], m(1)[:], bb, m(3)[:], alu.add, alu.add)
                    nc.sync.dma_start(y_d[rb, 0:TH], o21[:])
                elif j == 4:
                    v.scalar_tensor_tensor(
                        r["o11"][:], r["o11"][:], 0.0, m(4)[:], alu.add,
                        alu.subtract)
                    o12 = tl("o12")
                    v.scalar_tensor_tensor(
                        o12[:], m(2)[:], bt, m(4)[:], alu.add, alu.add)
                    nc.sync.dma_start(y_d[rt, TH:T_CORE], o12[:])
                elif j == 6:
                    v.scalar_tensor_tensor(
                        r["o11"][:], r["o11"][:], 0.0, m(6)[:], alu.add,
                        alu.add)
                    nc.sync.dma_start(y_d[rt, 0:TH], r["o11"][:])
                elif j == 5:
                    v.scalar_tensor_tensor(
                        r["o22"][:], r["o22"][:], 0.0, m(5)[:], alu.add,
                        alu.add)
                    nc.sync.dma_start(y_d[rb, TH:T_CORE], r["o22"][:])
                    for k in range(NPROD):
                        del stage[(k, obp)]
                    del rec[obp]

            # ---- startup group: j-major, c-outer across obp 0..G-1 ----
            for j in ORDER:
                ps_j = []
                for gob in range(G):
                    if j == 0 and gob == 0:
                        ps_j.append(ps_warm)
                    else:
                        ps_j.append(pp.tile(
                            [P, TH], dt.float32, tag="ps",
                            name=f"ps_{j}_{gob}"))
                for c in range(KH):
                    for gob in range(G):
                        nc.tensor.matmul(
                            ps_j[gob][:],
                            w_tiles[(j, gob)][:, c, :], xc_sb[j][:, c, :],
                            start=(c == 0), stop=(c == KH - 1),
                        )
                for gob in range(G):
                    evict(j, gob, ps_j[gob])
                for gob in range(G):
                    recombine_step(gob, j)
            for j, gob in list(w_tiles):
                if gob < G:
                    del w_tiles[(j, gob)]

            # ---- steady state: obp-major ----
            for obp in range(G, OBH):
                for j in ORDER:
                    if obp + 1 < OBH:
                        w_load(j, obp + 1,
                               eng=nc.scalar if (j + obp) % 2 else nc.gpsimd)
                    product(j, obp)
                    recombine_step(obp, j)

    nc.compile()
    return nc


def _get_nc():
    global _compiled
    if _compiled is None:
        _compiled = _build_nc()
    return _compiled


def _prep_inputs(x, W, b):
    x = np.asarray(x, dtype=np.float32)
    W = np.asarray(W, dtype=np.float32)
    b = np.asarray(b, dtype=np.float32)

    Wr = _matmul_hadU_np(W.astype(np.float64))  # [o, k] float64
    A11 = Wr[:HALF, :HALF]
    A12 = Wr[:HALF, HALF:]
    A21 = Wr[HALF:, :HALF]
    A22 = Wr[HALF:, HALF:]
    WCs = (A11 + A22, A21 + A22, A11, A22, A11 + A12, A21 - A11, A12 - A22)
    # pack[j][obp, p, c, jo] = WC_j[obp*128 + jo, c*128 + p]
    wc = np.stack([
        w.reshape(OBH, P, KH, P).transpose(0, 3, 2, 1) for w in WCs
    ]).astype(np.float16)
    wc = np.ascontiguousarray(wc)

    b_pack = np.ascontiguousarray(b.reshape(2 * OBH, P).T)  # [128, 32]

    xt = x.reshape(N_CORES, T_CORE, D).transpose(0, 2, 1)  # [core, k, t] f32
    B11 = xt[:, :HALF, :TH]
    B12 = xt[:, :HALF, TH:]
    B21 = xt[:, HALF:, :TH]
    B22 = xt[:, HALF:, TH:]
    XCs = (B11 + B22, B11, B12 - B22, B21 - B11, B22, B11 + B12, B21 + B22)
    # pack[core, j, p, c, t] = XC_j[core, c*128 + p, t]
    xc = np.stack([
        c.reshape(N_CORES, KH, P, TH).transpose(0, 2, 1, 3) for c in XCs
    ], axis=1).astype(np.float16)
    xc = np.ascontiguousarray(xc)

    in_maps = [
        {"xc": xc[i], "wc": wc, "bias": b_pack} for i in range(N_CORES)
    ]
    return in_maps


def _assemble(results):
    # yt per core: [4096 o, 1024 t] fp16 -> y[t, o] fp32
    parts = [r["yt"].T.astype(np.float32) for r in results]
    y = np.concatenate(parts, axis=0)  # [8192, 4096]
    return y.reshape(4, 2048, D)


def _run(x, W, b, **spmd_kwargs):
    from concourse.bass_utils import run_bass_kernel_spmd

    nc = _get_nc()
    in_maps = _prep_inputs(x, W, b)
    res = run_bass_kernel_spmd(nc, in_maps, list(range(N_CORES)), **spmd_kwargs)
    return _assemble(res.results), res


def kernel(x, W, b):
    out, _ = _run(x, W, b)
    return out


# revision 23
# speedup vs baseline: 1.0900x; 1.0317x over previous
"""Trainium2 Bass kernel for nn_InputRotationWrapper: y = WHT(x) @ W^T + b.

Algebraic fold: WHT (normalized Walsh-Hadamard along feature dim, H symmetric)
commutes into the weight: y = (x H) W^T = x (W H)^T.  The device runs a pure
GEMM  y = x @ Wr^T + b  with Wr = WHT(W) computed once on the host.

On top of the fold, one level of STRASSEN over 2x2x2 blocking of (o, k, t)
cuts the PE matmul count by 1/8 — the kernel is PE-streaming-bound at fp16
(1 moving column/cycle), so this is a direct 12.5% win that neither fp8
(accuracy: e4m3 x,W measures 3.8e-2 rel err vs the 2e-2 gate) nor uint8
(TRN2 silicon zeroes integer matmul products; probed via a NEFF dtype patch)
can reach.

  C = Wr @ x^T = [[C11 C12],[C21 C22]],  A = Wr halves, B = x^T halves
  M1=(A11+A22)(B11+B22) M2=(A21+A22)B11 M3=A11(B12-B22) M4=A22(B21-B11)
  M5=(A11+A12)B22 M6=(A21-A11)(B11+B12) M7=(A12-A22)(B21+B22)
  C11=M1+M4-M5+M7  C12=M3+M5  C21=M2+M4  C22=M1-M2+M3+M6

Per core (1024 tokens, data-parallel over 8 cores): 16 o-block iterations x
7 products x 16-chunk PSUM accumulation = 1792 matmuls of 512 cols (vs 2048
classical) ~ 387 us PE wall.  W-side combos are host-precomputed and
streamed per (product, o-block); x-side combos are SBUF-resident.

DMA economics (measured): one HWDGE queue sustains ~146 GB/s, the three
(gpsimd/scalar/sync) together ~300 GB/s, and a DMA trigger whose ring-WAR
semaphore isn't met blocks its whole queue.  The startup (x-combos + first
W) is the critical window, so:
  - only the 4 linearly independent x-combos are shipped (B11, B22,
    B12-B22, B21-B11 = 8.4 MB instead of 14.7); the other three are derived
    on the idle vector ALU: xc[M1]=B11+B22, xc[M6]=xc[M1]+xc[M3combo],
    xc[M7]=xc[M1]+xc[M4combo],
  - products run in ORDER=(M2,M5,M3,M4,M1,M6,M7) so shipped combos are
    consumed first and the derived ones are needed only from position 4,
  - W alternates gpsimd/scalar by position; x-combo c-halves ride
    scalar/sync ahead of same-position W; sync also carries bias+outputs,
  - the first G=4 o-blocks run j-major / c-outer so each arriving x chunk
    unlocks 4 matmuls during the DMA ramp (G=4 keeps two positions inside
    the 8-tile W ring; larger G ring-blocks next-position W),
  - ~40 dummy matmuls keep the PE HAM clock from re-throttling during the
    ramp.

Eviction: ScalarE copies each product PSUM->SBUF fp16; the vector ALU
recombines with scalar_tensor_tensor (bias fused via the per-partition
scalar operand) ACCUMULATING IN PLACE into the output tiles, so after the
last product of every o-block only evict -> one stt -> DMA remains.
"""
import sys

for _p in ("/opt/trn_rl_repo", "/root/.axon_site/_ro/trn_rl_repo"):
    if _p not in sys.path:
        sys.path.insert(0, _p)

import numpy as np

D = 4096          # feature dim (= rotation size)
TOKENS = 8192     # 4 * 2048
N_CORES = 8
T_CORE = TOKENS // N_CORES   # 1024 tokens per core
P = 128           # partitions
HALF = D // 2     # 2048: o/k half size
KH = HALF // P    # 16 contraction chunks per half
OBH = HALF // P   # 16 output blocks per half
TH = T_CORE // 2  # 512 tokens per t-half (= one matmul moving dim)
NPROD = 7

# product indices (m/M numbering): 0..6 = M1..M7
ORDER = (1, 4, 2, 3, 0, 5, 6)   # emission order; last = M7 -> 1-stt tail
SHIP = {1: 0, 4: 1, 2: 2, 3: 3}  # shipped x-combos -> slot in xc dram tensor

_compiled = None


def _matmul_hadU_np(x: np.ndarray) -> np.ndarray:
    """Normalized WHT along the last axis — exact port of the reference
    recursive-butterfly (K == 1 branch), in float64."""
    n = x.shape[-1]
    shape = x.shape
    v = x.reshape(-1, n, 1)
    while v.shape[1] > 1:
        b_, m, c = v.shape
        v = v.reshape(b_, m // 2, 2, c)
        a, b = v[:, :, 0, :], v[:, :, 1, :]
        v = np.concatenate([a + b, a - b], axis=-1)
    return v.reshape(shape) / np.sqrt(n)


def _build_nc():
    import concourse.tile as tile
    from concourse import bacc, mybir

    dt = mybir.dt
    alu = mybir.AluOpType
    nc = bacc.Bacc(None, target_bir_lowering=False)

    xc_d = nc.dram_tensor("xc", [4, P, KH, TH], dt.float16,
                          kind="ExternalInput")
    wc_d = nc.dram_tensor("wc", [NPROD, OBH, P, KH, P], dt.float16,
                          kind="ExternalInput")
    b_d = nc.dram_tensor("bias", [P, 2 * OBH], dt.float32,
                         kind="ExternalInput")
    y_d = nc.dram_tensor("yt", [D, T_CORE], dt.float16, kind="ExternalOutput")

    G = 4
    WRING = 8   # W tile ring (4 KB/partition each)
    MRING = 20  # staged-product ring (1 KB/partition each)
    ORING = 24  # output-tile ring (in-place accumulators live pos2..pos6)

    with tile.TileContext(nc) as tc:
        with (
            tc.tile_pool(name="xcp", bufs=1) as xcp,
            tc.tile_pool(name="wp", bufs=WRING) as wp,
            tc.tile_pool(name="mp", bufs=MRING) as mp,
            tc.tile_pool(name="op", bufs=ORING) as op,
            tc.tile_pool(name="bp", bufs=1) as bp,
            tc.tile_pool(name="pp", bufs=8, space="PSUM") as pp,
        ):
            b_sb = bp.tile([P, 2 * OBH], dt.float32)

            xc_sb = [
                xcp.tile([P, KH, TH], dt.float16, name=f"xc_{j}")
                for j in range(NPROD)
            ]

            dum = bp.tile([P, 256], dt.float16, tag="dum", name="dum")
            nc.vector.memset(dum[:], 0.0)

            w_tiles = {}

            def w_alloc(j, obp):
                t = wp.tile([P, KH, P], dt.float16, tag="w",
                            name=f"w_{j}_{obp}")
                w_tiles[(j, obp)] = t
                return t

            def w_load(j, obp, eng):
                t = w_alloc(j, obp)
                eng.dma_start(t[:], wc_d[j, obp, :, :, :])
                return t

            def xc_load(j, c0, n, eng=None):
                (eng or nc.scalar).dma_start(
                    xc_sb[j][:, c0:c0 + n, :], xc_d[SHIP[j], :, c0:c0 + n, :])

            # ---- DMA triggers in arrival-need order ----
            nc.sync.dma_start(b_sb[:], b_d[:])
            # pos0 (M2 <- B11) + its W, finely chunked for the DMA ramp
            xc_load(1, 0, 1)
            xc_load(1, 8, 2, eng=nc.sync)
            for gob in range(G):
                t = w_alloc(1, gob)
                nc.gpsimd.dma_start(t[:, 0:4, :], wc_d[1, gob, :, 0:4, :])
            xc_load(1, 1, 1)
            xc_load(1, 10, 2, eng=nc.sync)
            for gob in range(G):
                nc.gpsimd.dma_start(
                    w_tiles[(1, gob)][:, 4:8, :], wc_d[1, gob, :, 4:8, :])
            xc_load(1, 2, 2)
            xc_load(1, 12, 4, eng=nc.sync)
            for gob in range(G):
                nc.gpsimd.dma_start(
                    w_tiles[(1, gob)][:, 8:16, :], wc_d[1, gob, :, 8:16, :])
            xc_load(1, 4, 4)
            # pos1 (M5 <- B22), W on scalar
            xc_load(4, 0, 8)
            xc_load(4, 8, 8, eng=nc.sync)
            for gob in range(G):
                w_load(4, gob, nc.scalar)
            # pos2 (M3 <- B12-B22), W on gpsimd
            xc_load(2, 0, 8)
            xc_load(2, 8, 8, eng=nc.sync)
            for gob in range(G):
                w_load(2, gob, nc.gpsimd)
            # pos3 (M4 <- B21-B11), W on scalar
            xc_load(3, 0, 8)
            xc_load(3, 8, 8, eng=nc.sync)
            for gob in range(G):
                w_load(3, gob, nc.scalar)
            # pos4..6 W (their x-combos are derived on-device)
            for gob in range(G):
                w_load(0, gob, nc.gpsimd)
            for gob in range(G):
                w_load(5, gob, nc.scalar)
            for gob in range(G):
                w_load(6, gob, nc.gpsimd)

            # derive the dependent x-combos on the vector ALU:
            #   xc[M1] = B11+B22 = xc1+xc4
            #   xc[M6] = B11+B12 = xc[M1]+xc2
            #   xc[M7] = B21+B22 = xc[M1]+xc3
            nc.vector.scalar_tensor_tensor(
                xc_sb[0][:], xc_sb[1][:], 0.0, xc_sb[4][:], alu.add, alu.add)
            nc.vector.scalar_tensor_tensor(
                xc_sb[5][:], xc_sb[0][:], 0.0, xc_sb[2][:], alu.add, alu.add)
            nc.vector.scalar_tensor_tensor(
                xc_sb[6][:], xc_sb[0][:], 0.0, xc_sb[3][:], alu.add, alu.add)

            # ---- PE clock warmup through the DMA ramp ----
            ps_warm = pp.tile([P, TH], dt.float32, tag="ps", name="ps_w")
            for _ in range(40):
                nc.tensor.matmul(
                    ps_warm[:, 0:256], dum[:, 0:128], dum[:, 0:256],
                    start=True, stop=True,
                )

            stage = {}

            def evict(j, obp, ps):
                m = mp.tile([P, TH], dt.float16, tag="m", name=f"m_{j}_{obp}")
                nc.scalar.copy(m[:], ps[:])
                stage[(j, obp)] = m
                return m

            def product(j, obp, ps=None):
                if ps is None:
                    ps = pp.tile([P, TH], dt.float32, tag="ps",
                                 name=f"ps_{j}_{obp}")
                wt = w_tiles.pop((j, obp))
                for c in range(KH):
                    nc.tensor.matmul(
                        ps[:], wt[:, c, :], xc_sb[j][:, c, :],
                        start=(c == 0), stop=(c == KH - 1),
                    )
                evict(j, obp, ps)

            # Incremental in-place recombine on the vector ALU, keyed by the
            # just-finished product.  With ORDER=(1,4,2,3,0,5,6):
            #   pos2 (M3):  o12 = (M3+bt)+M5 ->DMA;  o22 = (M3+bb)-M2
            #   pos3 (M4):  o21 = (M2+bb)+M4 ->DMA;  o11 = (M4+bt)-M5
            #   pos4 (M1):  o11 += M1;  o22 += M1
            #   pos5 (M6):  o22 += M6 ->DMA
            #   pos6 (M7):  o11 += M7 ->DMA
            rec = {}

            def recombine_step(obp, j):
                bt = b_sb[:, obp:obp + 1]
                bb = b_sb[:, OBH + obp:OBH + obp + 1]
                m = lambda k: stage[(k, obp)]
                rt = slice(obp * P, (obp + 1) * P)
                rb = slice((OBH + obp) * P, (OBH + obp + 1) * P)
                r = rec.setdefault(obp, {})
                v = nc.vector

                def tl(nm):
                    return op.tile([P, TH], dt.float16, tag="o",
                                   name=f"{nm}_{obp}")

                if j == 2:
                    o12 = tl("o12")
                    v.scalar_tensor_tensor(
                        o12[:], m(2)[:], bt, m(4)[:], alu.add, alu.add)
                    nc.sync.dma_start(y_d[rt, TH:T_CORE], o12[:])
                    r["o22"] = tl("o22")
                    v.scalar_tensor_tensor(
                        r["o22"][:], m(2)[:], bb, m(1)[:], alu.add,
                        alu.subtract)
                elif j == 3:
                    o21 = tl("o21")
                    v.scalar_tensor_tensor(
                        o21[:], m(1)[:], bb, m(3)[:], alu.add, alu.add)
                    nc.sync.dma_start(y_d[rb, 0:TH], o21[:])
                    r["o11"] = tl("o11")
                    v.scalar_tensor_tensor(
                        r["o11"][:], m(3)[:], bt, m(4)[:], alu.add,
                        alu.subtract)
                elif j == 0:
                    v.scalar_tensor_tensor(
                        r["o11"][:], r["o11"][:], 0.0, m(0)[:], alu.add,
                        alu.add)
                    v.scalar_tensor_tensor(
                        r["o22"][:], r["o22"][:], 0.0, m(0)[:], alu.add,
                        alu.add)
                elif j == 5:
                    v.scalar_tensor_tensor(
                        r["o22"][:], r["o22"][:], 0.0, m(5)[:], alu.add,
                        alu.add)
                    nc.sync.dma_start(y_d[rb, TH:T_CORE], r["o22"][:])
                elif j == 6:
                    v.scalar_tensor_tensor(
                        r["o11"][:], r["o11"][:], 0.0, m(6)[:], alu.add,
                        alu.add)
                    nc.sync.dma_start(y_d[rt, 0:TH], r["o11"][:])
                    for k in range(NPROD):
                        del stage[(k, obp)]
                    del rec[obp]

            # ---- startup group: j-major, c-outer across obp 0..G-1 ----
            first = True
            for j in ORDER:
                ps_j = []
                for gob in range(G):
                    if first and gob == 0:
                        ps_j.append(ps_warm)
                    else:
                        ps_j.append(pp.tile(
                            [P, TH], dt.float32, tag="ps",
                            name=f"ps_{j}_{gob}"))
                first = False
                for c in range(KH):
                    for gob in range(G):
                        nc.tensor.matmul(
                            ps_j[gob][:],
                            w_tiles[(j, gob)][:, c, :], xc_sb[j][:, c, :],
                            start=(c == 0), stop=(c == KH - 1),
                        )
                for gob in range(G):
                    evict(j, gob, ps_j[gob])
                for gob in range(G):
                    recombine_step(gob, j)
            for j, gob in list(w_tiles):
                if gob < G:
                    del w_tiles[(j, gob)]

            # startup W for the first steady block
            for j in ORDER:
                w_load(j, G, nc.scalar if j % 2 else nc.gpsimd)

            # ---- steady state: obp-major ----
            for obp in range(G, OBH):
                for j in ORDER:
                    if obp + 1 < OBH:
                        w_load(j, obp + 1,
                               nc.scalar if (j + obp) % 2 else nc.gpsimd)
                    product(j, obp)
                    recombine_step(obp, j)

    nc.compile()
    return nc


def _get_nc():
    global _compiled
    if _compiled is None:
        _compiled = _build_nc()
    return _compiled


def _prep_inputs(x, W, b):
    x = np.asarray(x, dtype=np.float32)
    W = np.asarray(W, dtype=np.float32)
    b = np.asarray(b, dtype=np.float32)

    Wr = _matmul_hadU_np(W.astype(np.float64))  # [o, k] float64
    A11 = Wr[:HALF, :HALF]
    A12 = Wr[:HALF, HALF:]
    A21 = Wr[HALF:, :HALF]
    A22 = Wr[HALF:, HALF:]
    WCs = (A11 + A22, A21 + A22, A11, A22, A11 + A12, A21 - A11, A12 - A22)
    # pack[j][obp, p, c, jo] = WC_j[obp*128 + jo, c*128 + p]
    wc = np.stack([
        w.reshape(OBH, P, KH, P).transpose(0, 3, 2, 1) for w in WCs
    ]).astype(np.float16)
    wc = np.ascontiguousarray(wc)

    b_pack = np.ascontiguousarray(b.reshape(2 * OBH, P).T)  # [128, 32]

    xt = x.reshape(N_CORES, T_CORE, D).transpose(0, 2, 1)  # [core, k, t] f32
    B11 = xt[:, :HALF, :TH]
    B12 = xt[:, :HALF, TH:]
    B21 = xt[:, HALF:, :TH]
    B22 = xt[:, HALF:, TH:]
    # only the 4 independent combos are shipped (SHIP slots: M2,M5,M3,M4)
    XCs = (B11, B22, B12 - B22, B21 - B11)
    # pack[core, s, p, c, t] = XC_s[core, c*128 + p, t]
    xc = np.stack([
        c.reshape(N_CORES, KH, P, TH).transpose(0, 2, 1, 3) for c in XCs
    ], axis=1).astype(np.float16)
    xc = np.ascontiguousarray(xc)

    in_maps = [
        {"xc": xc[i], "wc": wc, "bias": b_pack} for i in range(N_CORES)
    ]
    return in_maps


def _assemble(results):
    # yt per core: [4096 o, 1024 t] fp16 -> y[t, o] fp32
    parts = [r["yt"].T.astype(np.float32) for r in results]
    y = np.concatenate(parts, axis=0)  # [8192, 4096]
    return y.reshape(4, 2048, D)


def _run(x, W, b, **spmd_kwargs):
    from concourse.bass_utils import run_bass_kernel_spmd

    nc = _get_nc()
    in_maps = _prep_inputs(x, W, b)
    res = run_bass_kernel_spmd(nc, in_maps, list(range(N_CORES)), **spmd_kwargs)
    return _assemble(res.results), res


def kernel(x, W, b):
    out, _ = _run(x, W, b)
    return out
